# revision 5
# baseline (speedup 1.0000x reference)
"""Bahdanau attention (nn_AttentionMechanism_12721693130824) on 8 TRN2 cores.

Data-parallel over batch: B=64 -> 8 batches per core. Single HBM read of the
encoder tensor per core (~16MB as bf16, transposed layout); the natural
layout needed by phase 2 is regenerated on-chip with the DMA xbar transpose.

Per core, per batch b:
  phase 1: projT = We.T @ enc[b].T on PE (bf16, fp32 accumulate), energyT =
           tanh(projT + bias) on ACT (bias = Wd.T dec + bd + be,
           per-partition), scores = Wa.T @ energyT on PE (f32r) into PSUM
           [1,4096] as 512-blocks.
  softmax: DVE evacuates score blocks with a permuted write pattern so that
           after a DRAM bounce the [128,32] tile holds s = 128*col + part,
           add mask bias (-1e10 where mask==0), exp on ACT (f32r copy for the
           attn output + bf16 copy for phase 2, accum_out partial sums),
           cross-partition total via ones matmul, reciprocal on DVE.
  phase 2: context = (unnormalized exp) @ enc[b] as 32 accumulating PE
           matmuls (bf16) whose rhs tiles come from dma_start_transpose of
           the resident encT tile (out[p, j, e'] = enc[s=128j+p, 128m+e']),
           scaled by 1/sum at evacuation. attn normalizes off the critical
           path via a free-dim replicate + DRAM-bounce broadcast of 1/sum.

mask and attn_out use a [128, 32] (s = 128*col + part) device layout; the
host permutes/un-permutes them (pure layout transforms). ba is skipped:
softmax is invariant to constant score shifts.
"""

import sys

for _p in ("/root/.axon_site/_ro/trn_rl_repo", "/opt/trn_rl_repo"):
    if _p not in sys.path:
        sys.path.append(_p)

import ml_dtypes
import numpy as np

import concourse.bass as bass
import concourse.tile as tile
from concourse import bacc, mybir
from concourse.bass_utils import run_bass_kernel_spmd

N_CORES = 8
B, S, DEC, ENC = 64, 4096, 512, 256
BL = B // N_CORES  # batches per core
P = 128
F32 = mybir.dt.float32
F32R = mybir.dt.float32r
BF16 = mybir.dt.bfloat16
I32 = mybir.dt.int32


def build_nc():
    nc = bacc.Bacc("TRN2", target_bir_lowering=False, debug=False,
                   num_devices=N_CORES)

    encT = nc.dram_tensor("encT", [BL, ENC, S], BF16, kind="ExternalInput").ap()
    mask = nc.dram_tensor("mask", [BL, P, S // P], I32, kind="ExternalInput").ap()
    decT = nc.dram_tensor("decT", [DEC, BL], F32, kind="ExternalInput").ap()
    Wd = nc.dram_tensor("Wd", [DEC, ENC], F32, kind="ExternalInput").ap()
    bd = nc.dram_tensor("bd", [ENC], F32, kind="ExternalInput").ap()
    We = nc.dram_tensor("We", [ENC, ENC], BF16, kind="ExternalInput").ap()
    be = nc.dram_tensor("be", [ENC], F32, kind="ExternalInput").ap()
    Wa = nc.dram_tensor("Wa", [ENC], F32R, kind="ExternalInput").ap()

    attn_out = nc.dram_tensor(
        "attn_out", [BL, P, S // P], F32R, kind="ExternalOutput").ap()
    ctx_out = nc.dram_tensor("ctx_out", [BL, ENC], F32, kind="ExternalOutput").ap()

    KD = DEC // P   # 4 contraction chunks for the decoder projection
    GF = ENC // P   # 2 feature chunks
    ME = ENC // P   # 2 encoder-feature contraction chunks
    HS = S // 2     # s-half processed per energy tile
    NC_ = S // P    # 32 context chunks

    with tile.TileContext(nc) as tc:
        with tc.tile_pool(name="singles", bufs=1) as singles, \
             tc.tile_pool(name="enct", bufs=2) as enct_pool, \
             tc.tile_pool(name="encnat", bufs=2) as encnat_pool, \
             tc.tile_pool(name="energy", bufs=2) as energy_pool, \
             tc.tile_pool(name="soft", bufs=4) as soft_pool, \
             tc.tile_pool(name="scrow", bufs=2) as scrow_pool, \
             tc.tile_pool(name="evac", bufs=4) as evac_pool, \
             tc.tile_pool(name="pp", bufs=2, space="PSUM") as pp_pool, \
             tc.tile_pool(name="scps", bufs=2, space="PSUM") as sc_pool, \
             tc.tile_pool(name="ctxps", bufs=2, space="PSUM") as ctx_pool, \
             tc.tile_pool(name="dramsc", bufs=2, space="DRAM") as dram_pool:

            # ---- one-time setup -------------------------------------------
            We_sb = singles.tile([P, ME, ENC], BF16)
            nc.sync.dma_start(out=We_sb, in_=We.rearrange("(m p) f -> p m f", p=P))
            Wa_sb = singles.tile([P, GF], F32R)
            nc.sync.dma_start(out=Wa_sb, in_=Wa.rearrange("(g p) -> p g", p=P))
            Wd_sb = singles.tile([P, KD, ENC], F32)
            nc.sync.dma_start(out=Wd_sb, in_=Wd.rearrange("(k p) f -> p k f", p=P))
            decT_sb = singles.tile([P, KD, BL], F32)
            nc.sync.dma_start(out=decT_sb, in_=decT.rearrange("(k p) b -> p k b", p=P))
            bd_sb = singles.tile([P, GF], F32)
            nc.sync.dma_start(out=bd_sb, in_=bd.rearrange("(g p) -> p g", p=P))
            be_sb = singles.tile([P, GF], F32)
            nc.sync.dma_start(out=be_sb, in_=be.rearrange("(g p) -> p g", p=P))
            ones_col = singles.tile([P, 1], F32)
            nc.vector.memset(ones_col, 1.0)
            ones_row = singles.tile([1, P], F32)
            nc.vector.memset(ones_row, 1.0)

            # biasT[f, b] = (dec @ Wd)[b, f] + bd[f] + be[f], f on partitions
            biasT_sb = singles.tile([P, GF, BL], F32)
            for g in range(GF):
                dps = pp_pool.tile([P, BL], F32, tag="pp")
                for k in range(KD):
                    nc.tensor.matmul(
                        dps,
                        Wd_sb[:, k, g * P:(g + 1) * P],
                        decT_sb[:, k, :],
                        start=(k == 0), stop=(k == KD - 1),
                    )
                nc.vector.tensor_scalar(
                    out=biasT_sb[:, g, :], in0=dps,
                    scalar1=bd_sb[:, g:g + 1], scalar2=be_sb[:, g:g + 1],
                    op0=mybir.AluOpType.add, op1=mybir.AluOpType.add,
                )

            # ---- per-batch pipeline ---------------------------------------
            for b in range(BL):
                encT_t = enct_pool.tile([P, ME, S], BF16)
                for m in range(ME):
                    nc.sync.dma_start(
                        out=encT_t[:, m, :], in_=encT[b, m * P:(m + 1) * P, :])

                # natural layout for phase 2 via on-chip xbar transpose:
                # enc_nat[p, j, m, e'] = enc[s = 128j + p, e = 128m + e']
                enc_nat = encnat_pool.tile([P, NC_, ME, P], BF16)
                for m in range(ME):
                    nc.scalar.dma_start_transpose(
                        out=enc_nat[:, :, m, :], in_=encT_t[:, m, :])

                mask_i = soft_pool.tile([P, S // P], I32)
                nc.sync.dma_start(out=mask_i, in_=mask[b])

                sc_row = scrow_pool.tile([1, S], F32)
                # row position of score s: 32*(s mod 128) + (s div 128)
                sc_row_v = sc_row.rearrange("p (i c) -> p i c", i=P)

                sc_dram = dram_pool.tile([1, S], F32)

                for h in range(2):
                    energy_t = energy_pool.tile([P, GF, HS], F32R)
                    for g in range(GF):
                        for u in range(HS // 1024):
                            pp = pp_pool.tile([P, 1024], F32, tag="pp")
                            for v in range(2):
                                sl = slice(h * HS + u * 1024 + v * 512,
                                           h * HS + u * 1024 + v * 512 + 512)
                                for m in range(ME):
                                    nc.tensor.matmul(
                                        pp[:, v * 512:(v + 1) * 512],
                                        We_sb[:, m, g * P:(g + 1) * P],
                                        encT_t[:, m, sl],
                                        start=(m == 0), stop=(m == ME - 1),
                                    )
                            nc.scalar.activation(
                                out=energy_t[:, g, u * 1024:(u + 1) * 1024],
                                in_=pp,
                                func=mybir.ActivationFunctionType.Tanh,
                                bias=biasT_sb[:, g, b:b + 1],
                            )
                    for k in range(HS // 512):  # score blocks in this half
                        sc_ps = sc_pool.tile([1, 512], F32)
                        for g in range(GF):
                            nc.tensor.matmul(
                                sc_ps,
                                Wa_sb[:, g:g + 1],
                                energy_t[:, g, k * 512:(k + 1) * 512],
                                start=(g == 0), stop=(g == GF - 1),
                            )
                        # permuted evacuation: element (i0, i1) of the block
                        # (s = 512*kk + 128*i1 + i0) lands at row position
                        # 4*kk + 32*i0 + i1
                        kk = h * (HS // 512) + k
                        nc.vector.tensor_copy(
                            out=sc_row_v[:, :, 4 * kk:4 * kk + 4],
                            in_=sc_ps.rearrange("p (i1 i0) -> p i0 i1", i1=4))

                nc.sync.dma_start(out=sc_dram, in_=sc_row)

                # softmax over [128, 32]; s = 128*c + p
                sc2 = soft_pool.tile([P, S // P], F32)
                nc.sync.dma_start(
                    out=sc2, in_=sc_dram.rearrange("q (p c) -> (q p) c", p=P))
                mbias = soft_pool.tile([P, S // P], F32)
                nc.vector.tensor_copy(out=mbias, in_=mask_i)
                nc.vector.tensor_scalar(
                    out=mbias, in0=mbias, scalar1=1e10, scalar2=-1e10,
                    op0=mybir.AluOpType.mult, op1=mybir.AluOpType.add)
                nc.vector.tensor_tensor(
                    out=sc2, in0=sc2, in1=mbias, op=mybir.AluOpType.add)
                expv = soft_pool.tile([P, S // P], F32R)
                colsum = soft_pool.tile([P, 1], F32)
                nc.scalar.activation(
                    out=expv, in_=sc2, func=mybir.ActivationFunctionType.Exp,
                    accum_out=colsum)
                expv_b = soft_pool.tile([P, S // P], BF16)
                nc.scalar.activation(
                    out=expv_b, in_=sc2, func=mybir.ActivationFunctionType.Exp)
                tot_ps = ctx_pool.tile([1, 1], F32, tag="ctx")
                nc.tensor.matmul(tot_ps, colsum, ones_col, start=True, stop=True)
                rcp = soft_pool.tile([1, 1], F32)
                nc.vector.reciprocal(out=rcp, in_=tot_ps)

                # phase 2 (critical path): context from unnormalized exp
                ctx_ps = ctx_pool.tile([1, ENC], F32, tag="ctx")
                for j in range(NC_):
                    nc.tensor.matmul(
                        ctx_ps,
                        expv_b[:, j:j + 1],
                        enc_nat[:, j, :, :],
                        start=(j == 0), stop=(j == NC_ - 1),
                    )
                ctx_sb = evac_pool.tile([1, ENC], F32, tag="ctxsb")
                nc.vector.tensor_scalar_mul(out=ctx_sb, in0=ctx_ps, scalar1=rcp)
                nc.sync.dma_start(
                    out=ctx_out[b].rearrange("(q e) -> q e", q=1), in_=ctx_sb)

                # attn output (off the critical path): broadcast 1/sum, scale
                r_rep = soft_pool.tile([1, P], F32)
                nc.vector.tensor_scalar_mul(out=r_rep, in0=ones_row, scalar1=rcp)
                r_dram = dram_pool.tile([1, P], F32, tag="rd")
                nc.sync.dma_start(out=r_dram, in_=r_rep)
                r_bc = soft_pool.tile([P, 1], F32)
                nc.sync.dma_start(
                    out=r_bc, in_=r_dram.rearrange("q (p c) -> (q p) c", p=P))
                attn2 = soft_pool.tile([P, S // P], F32R)
                nc.vector.tensor_scalar_mul(out=attn2, in0=expv, scalar1=r_bc)
                nc.sync.dma_start(out=attn_out[b], in_=attn2)

    nc.compile()
    return nc


_NC_CACHE = None


def _get_nc():
    global _NC_CACHE
    if _NC_CACHE is None:
        _NC_CACHE = build_nc()
    return _NC_CACHE


def _run(inputs, trace=False, tmpdir=None):
    decoder_hidden = np.asarray(inputs["decoder_hidden"], dtype=np.float32)
    encoder_outputs = np.asarray(inputs["encoder_outputs"], dtype=np.float32)
    mask = np.asarray(inputs["mask"], dtype=np.int32)
    Wd = np.asarray(inputs["Wd"], dtype=np.float32)
    bd = np.asarray(inputs["bd"], dtype=np.float32)
    We = np.asarray(inputs["We"], dtype=np.float32).astype(ml_dtypes.bfloat16)
    be = np.asarray(inputs["be"], dtype=np.float32)
    Wa = np.asarray(inputs["Wa"], dtype=np.float32)

    nc = _get_nc()
    in_maps = []
    for c in range(N_CORES):
        sl = slice(c * BL, (c + 1) * BL)
        encT_c = np.ascontiguousarray(
            encoder_outputs[sl].transpose(0, 2, 1)).astype(ml_dtypes.bfloat16)
        decT_c = np.ascontiguousarray(decoder_hidden[sl, 0, :].T)
        # device mask layout: [p, c] = mask[s = 128c + p]
        mask_c = np.ascontiguousarray(
            mask[sl].reshape(BL, S // P, P).transpose(0, 2, 1))
        in_maps.append({
            "encT": encT_c,
            "mask": mask_c,
            "decT": decT_c,
            "Wd": Wd, "bd": bd, "We": We, "be": be, "Wa": Wa,
        })
    kw = {}
    if trace:
        kw = dict(trace=True, tmpdir=tmpdir)
    res = run_bass_kernel_spmd(nc, in_maps, core_ids=list(range(N_CORES)), **kw)
    ctx = np.concatenate([res.results[c]["ctx_out"] for c in range(N_CORES)], axis=0)
    # un-permute attn: device [BL, p, c] -> attn[s = 128c + p]
    attn = np.concatenate(
        [res.results[c]["attn_out"].transpose(0, 2, 1).reshape(BL, S)
         for c in range(N_CORES)], axis=0)
    return (ctx[:, None, :].astype(np.float32), attn.astype(np.float32)), res


def kernel(**inputs):
    out, _ = _run(inputs, trace=False)
    return out


# revision 7
# speedup vs baseline: 1.1787x; 1.1787x over previous
"""Bahdanau attention (nn_AttentionMechanism_12721693130824) on 8 TRN2 cores.

Data-parallel over batch: B=64 -> 8 batches per core. Single HBM read of the
encoder tensor per core (~16MB as bf16, transposed layout); the natural
layout needed by phase 2 is regenerated on-chip with the DMA xbar transpose.

Per core, per batch b:
  phase 1: projT = We.T @ enc[b].T on PE (bf16, fp32 accumulate), energyT =
           tanh(projT + bias) on ACT (bias = Wd.T dec + bd + be,
           per-partition), scores = Wa.T @ energyT on PE (f32r) into PSUM
           [1,4096] as 512-blocks.
  softmax: DVE evacuates score blocks with a permuted write pattern so that
           after a DRAM bounce the [128,32] tile holds s = 128*col + part,
           add mask bias (-1e10 where mask==0), exp on ACT (f32r copy for the
           attn output + bf16 copy for phase 2, accum_out partial sums),
           cross-partition total via ones matmul, reciprocal on DVE.
  phase 2: context = (unnormalized exp) @ enc[b] as 32 accumulating PE
           matmuls (bf16) whose rhs tiles come from dma_start_transpose of
           the resident encT tile (out[p, j, e'] = enc[s=128j+p, 128m+e']),
           scaled by 1/sum at evacuation. attn normalizes off the critical
           path via a free-dim replicate + DRAM-bounce broadcast of 1/sum.

mask and attn_out use a [128, 32] (s = 128*col + part) device layout; the
host permutes/un-permutes them (pure layout transforms). ba is skipped:
softmax is invariant to constant score shifts.
"""

import sys

for _p in ("/root/.axon_site/_ro/trn_rl_repo", "/opt/trn_rl_repo"):
    if _p not in sys.path:
        sys.path.append(_p)

import ml_dtypes
import numpy as np

import concourse.bass as bass
import concourse.tile as tile
from concourse import bacc, mybir
from concourse.bass_utils import run_bass_kernel_spmd

N_CORES = 8
B, S, DEC, ENC = 64, 4096, 512, 256
BL = B // N_CORES  # batches per core
P = 128
F32 = mybir.dt.float32
F32R = mybir.dt.float32r
BF16 = mybir.dt.bfloat16
I32 = mybir.dt.int32


def build_nc():
    nc = bacc.Bacc("TRN2", target_bir_lowering=False, debug=False,
                   num_devices=N_CORES)

    encT = nc.dram_tensor("encT", [BL, ENC, S], BF16, kind="ExternalInput").ap()
    mask = nc.dram_tensor("mask", [BL, P, S // P], I32, kind="ExternalInput").ap()
    decT = nc.dram_tensor("decT", [DEC, BL], F32, kind="ExternalInput").ap()
    Wd = nc.dram_tensor("Wd", [DEC, ENC], F32, kind="ExternalInput").ap()
    bd = nc.dram_tensor("bd", [ENC], F32, kind="ExternalInput").ap()
    We = nc.dram_tensor("We", [ENC, ENC], BF16, kind="ExternalInput").ap()
    be = nc.dram_tensor("be", [ENC], F32, kind="ExternalInput").ap()
    Wa = nc.dram_tensor("Wa", [ENC], F32R, kind="ExternalInput").ap()

    attn_out = nc.dram_tensor(
        "attn_out", [BL, P, S // P], F32R, kind="ExternalOutput").ap()
    ctx_out = nc.dram_tensor("ctx_out", [BL, ENC], F32, kind="ExternalOutput").ap()

    KD = DEC // P   # 4 contraction chunks for the decoder projection
    GF = ENC // P   # 2 feature chunks
    ME = ENC // P   # 2 encoder-feature contraction chunks
    HS = S // 2     # s-half processed per energy tile
    NC_ = S // P    # 32 context chunks

    with tile.TileContext(nc) as tc:
        with tc.tile_pool(name="singles", bufs=1) as singles, \
             tc.tile_pool(name="enct", bufs=2) as enct_pool, \
             tc.tile_pool(name="encnat", bufs=2) as encnat_pool, \
             tc.tile_pool(name="energy", bufs=2) as energy_pool, \
             tc.tile_pool(name="soft", bufs=4) as soft_pool, \
             tc.tile_pool(name="scrow", bufs=2) as scrow_pool, \
             tc.tile_pool(name="evac", bufs=4) as evac_pool, \
             tc.tile_pool(name="pp", bufs=2, space="PSUM") as pp_pool, \
             tc.tile_pool(name="scps", bufs=2, space="PSUM") as sc_pool, \
             tc.tile_pool(name="ctxps", bufs=2, space="PSUM") as ctx_pool, \
             tc.tile_pool(name="dramsc", bufs=2, space="DRAM") as dram_pool:

            # ---- one-time setup -------------------------------------------
            We_sb = singles.tile([P, ME, ENC], BF16)
            nc.sync.dma_start(out=We_sb, in_=We.rearrange("(m p) f -> p m f", p=P))
            Wa_sb = singles.tile([P, GF], F32R)
            nc.sync.dma_start(out=Wa_sb, in_=Wa.rearrange("(g p) -> p g", p=P))
            Wd_sb = singles.tile([P, KD, ENC], F32)
            nc.sync.dma_start(out=Wd_sb, in_=Wd.rearrange("(k p) f -> p k f", p=P))
            decT_sb = singles.tile([P, KD, BL], F32)
            nc.sync.dma_start(out=decT_sb, in_=decT.rearrange("(k p) b -> p k b", p=P))
            bd_sb = singles.tile([P, GF], F32)
            nc.sync.dma_start(out=bd_sb, in_=bd.rearrange("(g p) -> p g", p=P))
            be_sb = singles.tile([P, GF], F32)
            nc.sync.dma_start(out=be_sb, in_=be.rearrange("(g p) -> p g", p=P))
            ones_col = singles.tile([P, 1], F32)
            nc.vector.memset(ones_col, 1.0)
            ones_row = singles.tile([1, P], F32)
            nc.vector.memset(ones_row, 1.0)

            # biasT[f, b] = (dec @ Wd)[b, f] + bd[f] + be[f], f on partitions
            biasT_sb = singles.tile([P, GF, BL], F32)
            for g in range(GF):
                dps = pp_pool.tile([P, BL], F32, tag="pp")
                for k in range(KD):
                    nc.tensor.matmul(
                        dps,
                        Wd_sb[:, k, g * P:(g + 1) * P],
                        decT_sb[:, k, :],
                        start=(k == 0), stop=(k == KD - 1),
                    )
                nc.vector.tensor_scalar(
                    out=biasT_sb[:, g, :], in0=dps,
                    scalar1=bd_sb[:, g:g + 1], scalar2=be_sb[:, g:g + 1],
                    op0=mybir.AluOpType.add, op1=mybir.AluOpType.add,
                )

            # ---- per-batch pipeline ---------------------------------------
            for b in range(BL):
                encT_t = enct_pool.tile([P, ME, S], BF16)
                for m in range(ME):
                    nc.sync.dma_start(
                        out=encT_t[:, m, :], in_=encT[b, m * P:(m + 1) * P, :])

                # natural layout for phase 2 via on-chip xbar transpose:
                # enc_nat[p, j, m, e'] = enc[s = 128j + p, e = 128m + e']
                enc_nat = encnat_pool.tile([P, NC_, ME, P], BF16)
                for m in range(ME):
                    nc.scalar.dma_start_transpose(
                        out=enc_nat[:, :, m, :], in_=encT_t[:, m, :])

                mask_i = soft_pool.tile([P, S // P], I32)
                nc.gpsimd.dma_start(out=mask_i, in_=mask[b])

                sc_row = scrow_pool.tile([1, S], F32)
                # row position of score s: 32*(s mod 128) + (s div 128)
                sc_row_v = sc_row.rearrange("p (i c) -> p i c", i=P)

                sc_dram = dram_pool.tile([1, S], F32)

                for h in range(2):
                    energy_t = energy_pool.tile([P, GF, HS], F32R)
                    for g in range(GF):
                        for u in range(HS // 1024):
                            pp = pp_pool.tile([P, 1024], F32, tag="pp")
                            for v in range(2):
                                sl = slice(h * HS + u * 1024 + v * 512,
                                           h * HS + u * 1024 + v * 512 + 512)
                                for m in range(ME):
                                    nc.tensor.matmul(
                                        pp[:, v * 512:(v + 1) * 512],
                                        We_sb[:, m, g * P:(g + 1) * P],
                                        encT_t[:, m, sl],
                                        start=(m == 0), stop=(m == ME - 1),
                                    )
                            nc.scalar.activation(
                                out=energy_t[:, g, u * 1024:(u + 1) * 1024],
                                in_=pp,
                                func=mybir.ActivationFunctionType.Tanh,
                                bias=biasT_sb[:, g, b:b + 1],
                            )
                    for k in range(HS // 512):  # score blocks in this half
                        sc_ps = sc_pool.tile([1, 512], F32)
                        for g in range(GF):
                            nc.tensor.matmul(
                                sc_ps,
                                Wa_sb[:, g:g + 1],
                                energy_t[:, g, k * 512:(k + 1) * 512],
                                start=(g == 0), stop=(g == GF - 1),
                            )
                        # permuted evacuation: element (i0, i1) of the block
                        # (s = 512*kk + 128*i1 + i0) lands at row position
                        # 4*kk + 32*i0 + i1
                        kk = h * (HS // 512) + k
                        nc.vector.tensor_copy(
                            out=sc_row_v[:, :, 4 * kk:4 * kk + 4],
                            in_=sc_ps.rearrange("p (i1 i0) -> p i0 i1", i1=4))

                nc.sync.dma_start(out=sc_dram, in_=sc_row)

                # softmax over [128, 32]; s = 128*c + p
                sc2 = soft_pool.tile([P, S // P], F32)
                nc.gpsimd.dma_start(
                    out=sc2, in_=sc_dram.rearrange("q (p c) -> (q p) c", p=P))
                mbias = soft_pool.tile([P, S // P], F32)
                nc.vector.tensor_copy(out=mbias, in_=mask_i)
                nc.vector.tensor_scalar(
                    out=mbias, in0=mbias, scalar1=1e10, scalar2=-1e10,
                    op0=mybir.AluOpType.mult, op1=mybir.AluOpType.add)
                nc.vector.tensor_tensor(
                    out=sc2, in0=sc2, in1=mbias, op=mybir.AluOpType.add)
                expv = soft_pool.tile([P, S // P], F32R)
                colsum = soft_pool.tile([P, 1], F32)
                nc.scalar.activation(
                    out=expv, in_=sc2, func=mybir.ActivationFunctionType.Exp,
                    accum_out=colsum)
                expv_b = soft_pool.tile([P, S // P], BF16)
                nc.scalar.activation(
                    out=expv_b, in_=sc2, func=mybir.ActivationFunctionType.Exp)
                tot_ps = ctx_pool.tile([1, 1], F32, tag="ctx")
                nc.tensor.matmul(tot_ps, colsum, ones_col, start=True, stop=True)
                rcp = soft_pool.tile([1, 1], F32)
                nc.vector.reciprocal(out=rcp, in_=tot_ps)

                # phase 2 (critical path): context from unnormalized exp
                ctx_ps = ctx_pool.tile([1, ENC], F32, tag="ctx")
                for j in range(NC_):
                    nc.tensor.matmul(
                        ctx_ps,
                        expv_b[:, j:j + 1],
                        enc_nat[:, j, :, :],
                        start=(j == 0), stop=(j == NC_ - 1),
                    )
                ctx_sb = evac_pool.tile([1, ENC], F32, tag="ctxsb")
                nc.vector.tensor_scalar_mul(out=ctx_sb, in0=ctx_ps, scalar1=rcp)
                nc.sync.dma_start(
                    out=ctx_out[b].rearrange("(q e) -> q e", q=1), in_=ctx_sb)

                # attn output (off the critical path): broadcast 1/sum with a
                # K=1 outer-product matmul (ones[1,128].T @ rcp[1,1]), scale
                rbc_ps = ctx_pool.tile([P, 1], F32, tag="ctx")
                nc.tensor.matmul(rbc_ps, ones_row, rcp, start=True, stop=True)
                r_bc = soft_pool.tile([P, 1], F32)
                nc.vector.tensor_copy(out=r_bc, in_=rbc_ps)
                attn2 = soft_pool.tile([P, S // P], F32R)
                nc.vector.tensor_scalar_mul(out=attn2, in0=expv, scalar1=r_bc)
                nc.gpsimd.dma_start(out=attn_out[b], in_=attn2)

    nc.compile()
    return nc


_NC_CACHE = None


def _get_nc():
    global _NC_CACHE
    if _NC_CACHE is None:
        _NC_CACHE = build_nc()
    return _NC_CACHE


def _run(inputs, trace=False, tmpdir=None):
    decoder_hidden = np.asarray(inputs["decoder_hidden"], dtype=np.float32)
    encoder_outputs = np.asarray(inputs["encoder_outputs"], dtype=np.float32)
    mask = np.asarray(inputs["mask"], dtype=np.int32)
    Wd = np.asarray(inputs["Wd"], dtype=np.float32)
    bd = np.asarray(inputs["bd"], dtype=np.float32)
    We = np.asarray(inputs["We"], dtype=np.float32).astype(ml_dtypes.bfloat16)
    be = np.asarray(inputs["be"], dtype=np.float32)
    Wa = np.asarray(inputs["Wa"], dtype=np.float32)

    nc = _get_nc()
    in_maps = []
    for c in range(N_CORES):
        sl = slice(c * BL, (c + 1) * BL)
        encT_c = np.ascontiguousarray(
            encoder_outputs[sl].transpose(0, 2, 1)).astype(ml_dtypes.bfloat16)
        decT_c = np.ascontiguousarray(decoder_hidden[sl, 0, :].T)
        # device mask layout: [p, c] = mask[s = 128c + p]
        mask_c = np.ascontiguousarray(
            mask[sl].reshape(BL, S // P, P).transpose(0, 2, 1))
        in_maps.append({
            "encT": encT_c,
            "mask": mask_c,
            "decT": decT_c,
            "Wd": Wd, "bd": bd, "We": We, "be": be, "Wa": Wa,
        })
    kw = {}
    if trace:
        kw = dict(trace=True, tmpdir=tmpdir)
    res = run_bass_kernel_spmd(nc, in_maps, core_ids=list(range(N_CORES)), **kw)
    ctx = np.concatenate([res.results[c]["ctx_out"] for c in range(N_CORES)], axis=0)
    # un-permute attn: device [BL, p, c] -> attn[s = 128c + p]
    attn = np.concatenate(
        [res.results[c]["attn_out"].transpose(0, 2, 1).reshape(BL, S)
         for c in range(N_CORES)], axis=0)
    return (ctx[:, None, :].astype(np.float32), attn.astype(np.float32)), res


def kernel(**inputs):
    out, _ = _run(inputs, trace=False)
    return out


# revision 8
# speedup vs baseline: 1.7877x; 1.5167x over previous
"""Bahdanau attention (nn_AttentionMechanism_12721693130824) on 8 TRN2 cores.

Data-parallel over batch: B=64 -> 8 batches per core. The encoder tensor is
read twice per core, both times as bf16 (~32MB total): once transposed
(phase 1: the feature contraction needs features on partitions) and once
natural (phase 2). Both host-side layout transforms give contiguous >=8KB
DMA partition lines, and the two streams ride different DMA queues (HWDGE
for the transposed read, SWDGE for the natural one).

Per core, per batch b:
  phase 1: projT = We.T @ enc[b].T on PE (bf16, fp32 accumulate), energyT =
           tanh(projT + bias) on ACT (bias = Wd.T dec + bd + be,
           per-partition), scores = Wa.T @ energyT on PE (f32r) into PSUM
           [1,4096] as 512-blocks.
  softmax: DVE evacuates score blocks into a [1,4096] row, DRAM bounce
           reshapes it to [128,32] (s = 32*part + col), add mask bias
           (-1e10 where mask==0), exp on ACT (f32r copy for the attn output
           + bf16 copy for phase 2, accum_out partial sums), cross-partition
           total via a ones matmul, reciprocal on DVE, and 1/sum broadcast
           to all partitions with a K=1 outer-product matmul.
  phase 2: context = (unnormalized exp) @ enc[b] as 32 accumulating PE
           matmuls (bf16) over the natural-layout copy, scaled by 1/sum at
           evacuation.

ba is skipped: softmax is invariant to constant score shifts.
"""

import sys

for _p in ("/root/.axon_site/_ro/trn_rl_repo", "/opt/trn_rl_repo"):
    if _p not in sys.path:
        sys.path.append(_p)

import ml_dtypes
import numpy as np

import concourse.bass as bass
import concourse.tile as tile
from concourse import bacc, mybir
from concourse.bass_utils import run_bass_kernel_spmd

N_CORES = 8
B, S, DEC, ENC = 64, 4096, 512, 256
BL = B // N_CORES  # batches per core
P = 128
F32 = mybir.dt.float32
F32R = mybir.dt.float32r
BF16 = mybir.dt.bfloat16
I32 = mybir.dt.int32


def build_nc():
    nc = bacc.Bacc("TRN2", target_bir_lowering=False, debug=False,
                   num_devices=N_CORES)

    encT = nc.dram_tensor("encT", [BL, ENC, S], BF16, kind="ExternalInput").ap()
    encN = nc.dram_tensor("encN", [BL, S, ENC], BF16, kind="ExternalInput").ap()
    mask = nc.dram_tensor("mask", [BL, S], I32, kind="ExternalInput").ap()
    decT = nc.dram_tensor("decT", [DEC, BL], F32, kind="ExternalInput").ap()
    Wd = nc.dram_tensor("Wd", [DEC, ENC], F32, kind="ExternalInput").ap()
    bd = nc.dram_tensor("bd", [ENC], F32, kind="ExternalInput").ap()
    We = nc.dram_tensor("We", [ENC, ENC], BF16, kind="ExternalInput").ap()
    be = nc.dram_tensor("be", [ENC], F32, kind="ExternalInput").ap()
    Wa = nc.dram_tensor("Wa", [ENC], F32R, kind="ExternalInput").ap()

    attn_out = nc.dram_tensor("attn_out", [BL, S], F32R, kind="ExternalOutput").ap()
    ctx_out = nc.dram_tensor("ctx_out", [BL, ENC], F32, kind="ExternalOutput").ap()

    KD = DEC // P   # 4 contraction chunks for the decoder projection
    GF = ENC // P   # 2 feature chunks
    ME = ENC // P   # 2 encoder-feature contraction chunks
    HS = S // 2     # s-half processed per energy tile
    NC_ = S // P    # 32 context chunks

    with tile.TileContext(nc) as tc:
        with tc.tile_pool(name="singles", bufs=1) as singles, \
             tc.tile_pool(name="enct", bufs=2) as enct_pool, \
             tc.tile_pool(name="encnat", bufs=2) as encnat_pool, \
             tc.tile_pool(name="energy", bufs=2) as energy_pool, \
             tc.tile_pool(name="soft", bufs=4) as soft_pool, \
             tc.tile_pool(name="scrow", bufs=2) as scrow_pool, \
             tc.tile_pool(name="evac", bufs=4) as evac_pool, \
             tc.tile_pool(name="pp", bufs=2, space="PSUM") as pp_pool, \
             tc.tile_pool(name="scps", bufs=2, space="PSUM") as sc_pool, \
             tc.tile_pool(name="ctxps", bufs=2, space="PSUM") as ctx_pool, \
             tc.tile_pool(name="dramsc", bufs=2, space="DRAM") as dram_pool:

            # ---- one-time setup -------------------------------------------
            We_sb = singles.tile([P, ME, ENC], BF16)
            nc.sync.dma_start(out=We_sb, in_=We.rearrange("(m p) f -> p m f", p=P))
            Wa_sb = singles.tile([P, GF], F32R)
            nc.sync.dma_start(out=Wa_sb, in_=Wa.rearrange("(g p) -> p g", p=P))
            Wd_sb = singles.tile([P, KD, ENC], F32)
            nc.sync.dma_start(out=Wd_sb, in_=Wd.rearrange("(k p) f -> p k f", p=P))
            decT_sb = singles.tile([P, KD, BL], F32)
            nc.sync.dma_start(out=decT_sb, in_=decT.rearrange("(k p) b -> p k b", p=P))
            bd_sb = singles.tile([P, GF], F32)
            nc.sync.dma_start(out=bd_sb, in_=bd.rearrange("(g p) -> p g", p=P))
            be_sb = singles.tile([P, GF], F32)
            nc.sync.dma_start(out=be_sb, in_=be.rearrange("(g p) -> p g", p=P))
            ones_col = singles.tile([P, 1], F32)
            nc.vector.memset(ones_col, 1.0)
            ones_row = singles.tile([1, P], F32)
            nc.vector.memset(ones_row, 1.0)

            # biasT[f, b] = (dec @ Wd)[b, f] + bd[f] + be[f], f on partitions
            biasT_sb = singles.tile([P, GF, BL], F32)
            for g in range(GF):
                dps = pp_pool.tile([P, BL], F32, tag="pp")
                for k in range(KD):
                    nc.tensor.matmul(
                        dps,
                        Wd_sb[:, k, g * P:(g + 1) * P],
                        decT_sb[:, k, :],
                        start=(k == 0), stop=(k == KD - 1),
                    )
                nc.vector.tensor_scalar(
                    out=biasT_sb[:, g, :], in0=dps,
                    scalar1=bd_sb[:, g:g + 1], scalar2=be_sb[:, g:g + 1],
                    op0=mybir.AluOpType.add, op1=mybir.AluOpType.add,
                )

            # ---- per-batch pipeline ---------------------------------------
            for b in range(BL):
                # natural layout (s = 32p + j) for phase 2 via SWDGE
                enc_nat = encnat_pool.tile([P, NC_, ENC], BF16)
                nc.gpsimd.dma_start(
                    out=enc_nat, in_=encN[b].rearrange("(p j) e -> p j e", p=P))

                encT_t = enct_pool.tile([P, ME, S], BF16)
                for m in range(ME):
                    nc.sync.dma_start(
                        out=encT_t[:, m, :], in_=encT[b, m * P:(m + 1) * P, :])

                mask_i = soft_pool.tile([P, S // P], I32)
                nc.gpsimd.dma_start(
                    out=mask_i, in_=mask[b].rearrange("(p c) -> p c", p=P))

                sc_row = scrow_pool.tile([1, S], F32)
                sc_dram = dram_pool.tile([1, S], F32)

                for h in range(2):
                    energy_t = energy_pool.tile([P, GF, HS], F32R)
                    for g in range(GF):
                        for u in range(HS // 1024):
                            pp = pp_pool.tile([P, 1024], F32, tag="pp")
                            for v in range(2):
                                sl = slice(h * HS + u * 1024 + v * 512,
                                           h * HS + u * 1024 + v * 512 + 512)
                                for m in range(ME):
                                    nc.tensor.matmul(
                                        pp[:, v * 512:(v + 1) * 512],
                                        We_sb[:, m, g * P:(g + 1) * P],
                                        encT_t[:, m, sl],
                                        start=(m == 0), stop=(m == ME - 1),
                                    )
                            nc.scalar.activation(
                                out=energy_t[:, g, u * 1024:(u + 1) * 1024],
                                in_=pp,
                                func=mybir.ActivationFunctionType.Tanh,
                                bias=biasT_sb[:, g, b:b + 1],
                            )
                    for k in range(HS // 512):  # score blocks in this half
                        sc_ps = sc_pool.tile([1, 512], F32)
                        for g in range(GF):
                            nc.tensor.matmul(
                                sc_ps,
                                Wa_sb[:, g:g + 1],
                                energy_t[:, g, k * 512:(k + 1) * 512],
                                start=(g == 0), stop=(g == GF - 1),
                            )
                        kk = h * (HS // 512) + k
                        nc.vector.tensor_copy(
                            out=sc_row[:, kk * 512:(kk + 1) * 512], in_=sc_ps)

                nc.sync.dma_start(out=sc_dram, in_=sc_row)

                # softmax over [128, 32]; s = 32p + c
                sc2 = soft_pool.tile([P, S // P], F32)
                nc.gpsimd.dma_start(
                    out=sc2, in_=sc_dram.rearrange("q (p c) -> (q p) c", p=P))
                mbias = soft_pool.tile([P, S // P], F32)
                nc.vector.tensor_copy(out=mbias, in_=mask_i)
                nc.vector.tensor_scalar(
                    out=mbias, in0=mbias, scalar1=1e10, scalar2=-1e10,
                    op0=mybir.AluOpType.mult, op1=mybir.AluOpType.add)
                nc.vector.tensor_tensor(
                    out=sc2, in0=sc2, in1=mbias, op=mybir.AluOpType.add)
                expv = soft_pool.tile([P, S // P], F32R)
                colsum = soft_pool.tile([P, 1], F32)
                nc.scalar.activation(
                    out=expv, in_=sc2, func=mybir.ActivationFunctionType.Exp,
                    accum_out=colsum)
                expv_b = soft_pool.tile([P, S // P], BF16)
                nc.scalar.activation(
                    out=expv_b, in_=sc2, func=mybir.ActivationFunctionType.Exp)
                tot_ps = ctx_pool.tile([1, 1], F32, tag="ctx")
                nc.tensor.matmul(tot_ps, colsum, ones_col, start=True, stop=True)
                rcp = soft_pool.tile([1, 1], F32)
                nc.vector.reciprocal(out=rcp, in_=tot_ps)

                # phase 2 (critical path): context from unnormalized exp
                ctx_ps = ctx_pool.tile([1, ENC], F32, tag="ctx")
                for j in range(NC_):
                    nc.tensor.matmul(
                        ctx_ps,
                        expv_b[:, j:j + 1],
                        enc_nat[:, j, :],
                        start=(j == 0), stop=(j == NC_ - 1),
                    )
                ctx_sb = evac_pool.tile([1, ENC], F32, tag="ctxsb")
                nc.vector.tensor_scalar_mul(out=ctx_sb, in0=ctx_ps, scalar1=rcp)
                nc.sync.dma_start(
                    out=ctx_out[b].rearrange("(q e) -> q e", q=1), in_=ctx_sb)

                # attn output (off the critical path): broadcast 1/sum with a
                # K=1 outer-product matmul (ones[1,128].T @ rcp[1,1]), scale
                rbc_ps = ctx_pool.tile([P, 1], F32, tag="ctx")
                nc.tensor.matmul(rbc_ps, ones_row, rcp, start=True, stop=True)
                r_bc = soft_pool.tile([P, 1], F32)
                nc.vector.tensor_copy(out=r_bc, in_=rbc_ps)
                attn2 = soft_pool.tile([P, S // P], F32R)
                nc.vector.tensor_scalar_mul(out=attn2, in0=expv, scalar1=r_bc)
                nc.gpsimd.dma_start(
                    out=attn_out[b].rearrange("(p c) -> p c", p=P), in_=attn2)

    nc.compile()
    return nc


_NC_CACHE = None


def _get_nc():
    global _NC_CACHE
    if _NC_CACHE is None:
        _NC_CACHE = build_nc()
    return _NC_CACHE


def _run(inputs, trace=False, tmpdir=None):
    decoder_hidden = np.asarray(inputs["decoder_hidden"], dtype=np.float32)
    encoder_outputs = np.asarray(inputs["encoder_outputs"], dtype=np.float32)
    mask = np.asarray(inputs["mask"], dtype=np.int32)
    Wd = np.asarray(inputs["Wd"], dtype=np.float32)
    bd = np.asarray(inputs["bd"], dtype=np.float32)
    We = np.asarray(inputs["We"], dtype=np.float32).astype(ml_dtypes.bfloat16)
    be = np.asarray(inputs["be"], dtype=np.float32)
    Wa = np.asarray(inputs["Wa"], dtype=np.float32)

    nc = _get_nc()
    in_maps = []
    for c in range(N_CORES):
        sl = slice(c * BL, (c + 1) * BL)
        enc_c = encoder_outputs[sl]
        encT_c = np.ascontiguousarray(
            enc_c.transpose(0, 2, 1)).astype(ml_dtypes.bfloat16)
        encN_c = np.ascontiguousarray(enc_c).astype(ml_dtypes.bfloat16)
        decT_c = np.ascontiguousarray(decoder_hidden[sl, 0, :].T)
        in_maps.append({
            "encT": encT_c,
            "encN": encN_c,
            "mask": np.ascontiguousarray(mask[sl]),
            "decT": decT_c,
            "Wd": Wd, "bd": bd, "We": We, "be": be, "Wa": Wa,
        })
    kw = {}
    if trace:
        kw = dict(trace=True, tmpdir=tmpdir)
    res = run_bass_kernel_spmd(nc, in_maps, core_ids=list(range(N_CORES)), **kw)
    ctx = np.concatenate([res.results[c]["ctx_out"] for c in range(N_CORES)], axis=0)
    attn = np.concatenate([res.results[c]["attn_out"] for c in range(N_CORES)], axis=0)
    return (ctx[:, None, :].astype(np.float32), attn.astype(np.float32)), res


def kernel(**inputs):
    out, _ = _run(inputs, trace=False)
    return out
